# revision 1
# baseline (speedup 1.0000x reference)
"""Trainium2 Bass kernel for nn_Align: batched quaternion->rotmat + rigid transform.

reference math (per structure j of 64):
    q = (1, b, c, d) / sqrt(s),  s = 1 + b^2 + c^2 + d^2
    R = rotmat(q)                       # 3x3
    out[j] = pred[j] @ R + t[j]         # [91,3] @ [3,3] + [3]

Sharding: data-parallel over the 8 NeuronCores, 8 structures per core.

Per-core layout: partitions = (structure j:8, point-group g:13) = 104,
free dim = (point-in-group q:7, coord m:3) = 21.  R = N * (2/s) - I with
numerators N assembled from one broadcast-AP product op over the packed
row tail rc = [1 b c d b c]:  P[:, a+4b] = rc_a * rc_{a+b} gives
[1 bb cc dd | b bc cd db], so sum(P[0:4]) = s and the off-diagonal
products sit contiguously.  The transform runs as 9 fused
per-partition-scalar multiply-adds (3 per output coordinate).

Raw Bass (no Tile: this walrus build encodes at most one sync-wait per
compute instruction).  Every DVE RAW dep is semaphore-synced (streaming
same-engine RAW is not safe on HW), and the kernel clears its semaphores
then all-engine-barriers before use (sems persist across NEFF runs).
"""

import numpy as np

NCORES = 8
J = 8         # structures per core
G = 13        # point groups per structure
Q = 7         # points per group  (G*Q = 91)
PARTS = J * G  # 104 partitions

# R-tile column layout: [0:3]=diag(R00,R11,R22) [3:6]=plus(R10,R21,R02)
# [6:9]=minus(R01,R12,R20).  Columns holding (R[0,n], R[1,n], R[2,n]):
CHANNEL_COLS = {0: (0, 3, 8), 1: (6, 1, 4), 2: (5, 7, 2)}

_cache = {}


def _build_nc():
    import dataclasses

    import concourse.bass as bass
    import concourse.mybir as mybir

    f32 = mybir.dt.float32
    Alu = mybir.AluOpType

    nc = bass.Bass()
    # host-packed per (structure, point-group) row (30 floats):
    # [ 21 pred floats (7 points x 3 coords) | 1 b c d b c t0 t1 t2 ]
    packed = nc.dram_tensor("packed", [PARTS, 30], f32, kind="ExternalInput")
    out = nc.dram_tensor("out", [J, 91, 3], f32, kind="ExternalOutput")

    with (
        nc.sbuf_tensor([PARTS, 30], f32) as PK_t,
        nc.sbuf_tensor([PARTS, 8], f32) as P_t,
        nc.sbuf_tensor([PARTS, 9], f32) as R_t,
        nc.sbuf_tensor([PARTS, 1], f32) as S2_t,
        nc.sbuf_tensor([PARTS, 1], f32) as INV_t,
        nc.sbuf_tensor([PARTS, 6 * Q], f32) as ACC_t,
        nc.sbuf_tensor([PARTS, 21], f32) as O_t,
        nc.semaphore("dma_in") as dma_in_sem,
        nc.semaphore("v") as v_sem,
        nc.semaphore("dve_done") as dve_sem,
        nc.semaphore("dma_out") as dma_out_sem,
        nc.Block() as block,
    ):
        PK = PK_t[:, :]
        P = P_t[:, :]
        R = R_t[:, :]
        S2 = S2_t[:, :]
        INV = INV_t[:, :]
        O = O_t[:, :]
        ACC = [ACC_t[:, i * Q:(i + 1) * Q] for i in range(6)]
        RT = PK[:, 21:30]   # [1 b c d b c t0 t1 t2]

        def _pseudo_barrier(eng):
            # NRT expands this to a real all-engine barrier on runtime
            # semaphores outside the kernel sem range — stale-state proof.
            eng.isa(
                nc.isa.Opcode.NEURON_ISA_TPB_OPCODE_PSEUDO_SYNC_BARRIER,
                {},
                struct_name="NEURON_ISA_TPB_UNKNOWN_STRUCT",
                verify=False,
            )

        @block.gpsimd
        def _(gpsimd):
            # Stale-semaphore preamble: semaphores are NOT reset between NEFF
            # executions, and waits here use absolute values.  Clear every sem
            # this kernel waits on or increments, THEN barrier — without the
            # barrier an engine can pass its first wait on a stale value
            # before the clear lands (observed as a HW deadlock).  The Block
            # exit barrier's event sems (nc.barrier_sems) are self-managed
            # and were never cleared by the framework preamble either.
            nums = sorted(x.num for x in (dma_in_sem, v_sem, dve_sem, dma_out_sem))
            assert nums[-1] - nums[0] == 3, nums
            r = range(nums[0], nums[-1] + 1)
            gpsimd.dma_reset(r)
            gpsimd.sem_clear(r)
            _pseudo_barrier(gpsimd)

        @block.scalar
        def _(scalar):
            _pseudo_barrier(scalar)

        @block.tensor
        def _(tensor):
            _pseudo_barrier(tensor)

        @block.sync
        def _(sync):
            _pseudo_barrier(sync)
            sync.dma_start(out=PK, in_=packed[:, :]).then_inc(dma_in_sem, 16)
            sync.wait_ge(dve_sem, 1)
            sync.dma_start(
                out=out[:, :, :].rearrange("j (g q) m -> (j g) (q m)", g=G),
                in_=O,
            ).then_inc(dma_out_sem, 16)
            sync.wait_ge(dma_out_sem, 16)

        @block.vector
        def _(vector):
            _pseudo_barrier(vector)
            vector.wait_ge(dma_in_sem, 16)

            # DVE streaming RAW is not safe without sem sync (HW-verified):
            # every op bumps v_sem; consumers wait on the cumulative count.
            def op(k, *args, **kw):
                return getattr(vector, k)(*args, **kw).then_inc(v_sem, 1)

            # ---- rotation matrix ----
            # P[:, a+4b] = u_a * u_{a+b} over rc = RT[0:6] = [1 b c d b c],
            # b in {0,1}, a in {0,3}:
            #   b=0 -> [1 bb cc dd]   (cols 0:4; sum = s)
            #   b=1 -> [b bc cd db]   (cols 4:8; bc,cd,db at 5:8)
            u_ap = RT[:, 0:4].unsqueeze(1).broadcast_to([PARTS, 2, 4])
            v_base = RT[:, 0:4].unsqueeze(1).broadcast_to([PARTS, 2, 4])
            pairs = [list(p) for p in v_base.ap]
            pairs[1][0] = 1  # dims [partition, b, a]; b-step 1 elem -> u_{a+b}
            v_ap = dataclasses.replace(v_base, ap=pairs)
            p_out = P.rearrange("p (b a) -> p b a", b=2)
            op("tensor_tensor", out=p_out, in0=u_ap, in1=v_ap, op=Alu.mult)  # 1
            vector.wait_ge(v_sem, 1)
            op("reduce_sum", out=S2, in_=P[:, 0:4],                          # 2  s
               axis=mybir.AxisListType.X)
            vector.wait_ge(v_sem, 2)
            op("reciprocal", out=INV, in_=S2)                                # 3  1/s
            # numerators: diag = P[1:4]+1;  plus/minus = [bc,cd,db] -+ [d,b,c]
            op("tensor_scalar", out=R[:, 0:3], in0=P[:, 1:4], scalar1=1.0,   # 4
               scalar2=None, op0=Alu.add)
            op("tensor_tensor", out=R[:, 3:6], in0=P[:, 5:8],                # 5
               in1=RT[:, 3:6], op=Alu.add)
            op("tensor_tensor", out=R[:, 6:9], in0=P[:, 5:8],                # 6
               in1=RT[:, 3:6], op=Alu.subtract)
            vector.wait_ge(v_sem, 6)
            op("tensor_scalar", out=R, in0=R, scalar1=INV, scalar2=2.0,      # 7
               op0=Alu.mult, op1=Alu.mult)                                   #   R=num*2/s
            vector.wait_ge(v_sem, 7)
            op("tensor_scalar", out=R[:, 0:3], in0=R[:, 0:3], scalar1=-1.0,  # 8
               scalar2=None, op0=Alu.add)                                    #   diag -1

            # ---- transform (channel-interleaved) ----
            xm = PK[:, 0:21].rearrange("p (q m) -> p m q", m=3)
            om = O.rearrange("p (q m) -> p m q", m=3)
            a0 = [ACC[2 * n][:, :] for n in range(3)]
            a1 = [ACC[2 * n + 1][:, :] for n in range(3)]
            vector.wait_ge(v_sem, 8)
            for n in range(3):        # 9,10,11:  I1_n = X0*R[0,n] + t_n
                c0 = CHANNEL_COLS[n][0]
                op("tensor_scalar", out=a0[n], in0=xm[:, 0, :],
                   scalar1=R[:, c0:c0 + 1], scalar2=RT[:, 6 + n:7 + n],
                   op0=Alu.mult, op1=Alu.add)
            for n in range(3):        # 12,13,14:  I2_n = X1*R[1,n] + I1_n
                vector.wait_ge(v_sem, 9 + n)
                c1 = CHANNEL_COLS[n][1]
                op("scalar_tensor_tensor", out=a1[n], in0=xm[:, 1, :],
                   scalar=R[:, c1:c1 + 1], in1=a0[n],
                   op0=Alu.mult, op1=Alu.add)
            for n in range(3):        # 15,16,17:  out_n = X2*R[2,n] + I2_n
                vector.wait_ge(v_sem, 12 + n)
                c2 = CHANNEL_COLS[n][2]
                ins = vector.scalar_tensor_tensor(
                    out=om[:, n, :], in0=xm[:, 2, :],
                    scalar=R[:, c2:c2 + 1], in1=a1[n],
                    op0=Alu.mult, op1=Alu.add,
                )
                if n < 2:
                    ins.then_inc(v_sem, 1)
                else:
                    ins.then_inc(dve_sem, 1)

    return nc


def get_nc():
    if "nc" not in _cache:
        _cache["nc"] = _build_nc()
    return _cache["nc"]


def shard_inputs(pred_coor, r_vector, t_vector):
    # packed per (structure, group) row: [21 pred | 1 b c d b c | t0 t1 t2]
    n = pred_coor.shape[0]
    pk = np.empty((n, G, 30), dtype=np.float32)
    pk[:, :, 0:21] = pred_coor.reshape(n, G, 21)
    pk[:, :, 21] = 1.0
    pk[:, :, 22:25] = r_vector[:, None, :]
    pk[:, :, 25:27] = r_vector[:, None, 0:2]
    pk[:, :, 27:30] = t_vector[:, None, :]
    pk = pk.reshape(n * G, 30)
    return [
        {"packed": np.ascontiguousarray(pk[c * PARTS : (c + 1) * PARTS])}
        for c in range(NCORES)
    ]


def run(pred_coor, r_vector, t_vector, trace=False):
    from concourse.bass_utils import run_bass_kernel_spmd

    nc = get_nc()
    in_maps = shard_inputs(pred_coor, r_vector, t_vector)
    res = run_bass_kernel_spmd(nc, in_maps, list(range(NCORES)), trace=trace)
    full = np.concatenate([res.results[c]["out"] for c in range(NCORES)], axis=0)
    return full, res


def kernel(pred_coor, r_vector, t_vector):
    pred_coor = np.asarray(pred_coor, dtype=np.float32)
    r_vector = np.asarray(r_vector, dtype=np.float32)
    t_vector = np.asarray(t_vector, dtype=np.float32)
    full, _ = run(pred_coor, r_vector, t_vector, trace=False)
    return full



# revision 2
# speedup vs baseline: 1.1353x; 1.1353x over previous
"""Trainium2 Bass kernel for nn_Align — v7.

reference math (per structure j of 64):
    q = (1, b, c, d) / sqrt(s),  s = 1 + b^2 + c^2 + d^2
    R = rotmat(q);  out[j] = pred[j] @ R + t[j]

Sharding: data-parallel over the 8 NeuronCores, 8 structures per core.
Per-core layout: partitions = (structure j:8, group g:13) = 104, free dim
= (point q:7, coord n:3) = 21.

Formulation: with V = (b,c,d), numerators N (row-major S[3m+n]) obey
    N[m,n] = V[m]*V[n] + A[m,n],   A = [[1,-d,c],[d,1,-b],[-c,b,1]]
and with Ns = N*(2/s) = R+I:  out = X @ Ns + (t - X).
The product matrix is ONE broadcast outer-product op reading just 3 cols;
it lands in a PK scratch range whose stride-4 diagonal, together with a
host-packed 1.0 four columns past the last diag slot, forms the s-reduce
input (s = 1+bb+cc+dd) — no separate product packing needed.

8 DVE ops:
  1 P9 = V[m]*V[n] -> PK[36:45]   2 s = PK[36:49:4] = bb+cc+dd+1
  3 INV = 1/s                     4 S = P9 + A9 = N
  5 S *= 2/s  (Ns = R+I)          6 D = t-X -> W[...,3]
  7 W[q,n,m] = X[q,m]*Ns[m,n]     8 O[q,n] = sum_m W[q,n,0:4]

Host packs per (j,g) row (49 floats):
  [pred 21 | b c d | A9 = 1 -d c d 1 -b -c b 1 | t0 t1 t2 |
   9 zero scratch | 0 0 0 | 1.0]

Stale-semaphore safety (sems persist across NEFF runs): each sem is
cleared by its WAITER engine via an InstEventSemaphore write-0 before
that engine's first wait (DVE: dma_in+v, SP: dve+dma_out).  The framework
preamble ends in an all-engine barrier, so the clears are start-
synchronized, and every increment of a cleared sem is >=1us away (a DMA
completion or the whole DVE chain), so a clear cannot erase a live inc.
SP's clears sit after the input dma_start, hidden in its ~2.2us shadow.

No nc.Block(): no exit all-engine barrier.  SP's final wait on dma_out
keeps the NEFF alive until the output write lands; other engines idle.
Every DVE RAW dep is semaphore-synced (streaming same-engine RAW is not
safe on HW); deps transitively covered by an earlier-stalled wait carry
no encoded wait.
"""

import numpy as np

NCORES = 8
J = 8
G = 13
Q = 7
PARTS = J * G
ROW = 49

_cache = {}


def _build_nc():
    import dataclasses

    import concourse.bass as bass
    import concourse.mybir as mybir

    f32 = mybir.dt.float32
    Alu = mybir.AluOpType

    def restride(ap, dim_strides):
        pairs = [list(p) for p in ap.ap]
        for i, s in dim_strides.items():
            pairs[i][0] = s
        return dataclasses.replace(ap, ap=pairs)

    def sem_write0(engine, sem):
        # InstEventSemaphore: always-true wait carrying a write-to-0 update.
        # The natural sequencer encoding of "set semaphore := 0" on any engine.
        inst = engine.wait_ge(sem, 0)
        inst.ins.sync_info.on_update.append(
            mybir.SyncUpdate(
                sync_type="semaphore", id=sem.num, ant_name=sem.name,
                update_mode="sem-wr-imm", update_value=0,
            )
        )
        return inst

    nc = bass.Bass()
    packed = nc.dram_tensor("packed", [PARTS, ROW], f32, kind="ExternalInput")
    out = nc.dram_tensor("out", [J, 91, 3], f32, kind="ExternalOutput")

    with (
        nc.sbuf_tensor([PARTS, ROW], f32) as PK_t,
        nc.sbuf_tensor([PARTS, 9], f32) as S_t,
        nc.sbuf_tensor([PARTS, 1], f32) as S2_t,
        nc.sbuf_tensor([PARTS, 1], f32) as INV_t,
        nc.sbuf_tensor([PARTS, Q * 3 * 4], f32) as W_t,
        nc.sbuf_tensor([PARTS, 21], f32) as O_t,
        nc.semaphore("dma_in") as dma_in_sem,
        nc.semaphore("v") as v_sem,
        nc.semaphore("dve_done") as dve_sem,
        nc.semaphore("dma_out") as dma_out_sem,
    ):
        PK = PK_t[:, :]
        S = S_t[:, :]
        S2 = S2_t[:, :]
        INV = INV_t[:, :]
        O = O_t[:, :]
        V3 = PK[:, 21:24]     # (b, c, d)
        A9 = PK[:, 24:33]     # 1 -d c d 1 -b -c b 1
        T3 = PK[:, 33:36]     # t
        P9 = PK[:, 36:45]     # outer-product scratch (host zeros)
        # s-reduce input: (bb, cc, dd, 1.0) at cols 36, 40, 44, 48

        sync = nc.sync
        vector = nc.vector

        # ---- SP stream ----
        sync.dma_start(out=PK, in_=packed[:, :]).then_inc(dma_in_sem, 16)
        sem_write0(sync, dve_sem)
        sem_write0(sync, dma_out_sem)
        sync.wait_ge(dve_sem, 1)
        sync.dma_start(
            out=out[:, :, :].rearrange("j (g q) m -> (j g) (q m)", g=G),
            in_=O,
        ).then_inc(dma_out_sem, 16)
        sync.wait_ge(dma_out_sem, 16)

        # ---- DVE stream ----
        sem_write0(vector, dma_in_sem)
        sem_write0(vector, v_sem)
        vector.wait_ge(dma_in_sem, 16)

        def op(k, *args, **kw):
            return getattr(vector, k)(*args, **kw).then_inc(v_sem, 1)

        # 1: P9[3m+n] = V[m] * V[n]
        Vm = restride(V3.unsqueeze(2).broadcast_to([PARTS, 3, 3]), {2: 0})
        Vn = restride(V3.unsqueeze(1).broadcast_to([PARTS, 3, 3]), {1: 0})
        op("tensor_tensor", out=P9.rearrange("p (m n) -> p m n", m=3),
           in0=Vm, in1=Vn, op=Alu.mult)
        vector.wait_ge(v_sem, 1)
        # 2: s = bb + cc + dd + 1  (stride-4 over PK cols 36,40,44,48)
        op("reduce_sum", out=S2, in_=restride(PK[:, 36:40], {1: 4}),
           axis=mybir.AxisListType.X)
        vector.wait_ge(v_sem, 2)
        # 3: INV = 1/s
        op("reciprocal", out=INV, in_=S2)
        # 4: S = P9 + A9 = N   (dep op1, covered by op2's wait)
        op("tensor_tensor", out=S, in0=P9, in1=A9, op=Alu.add)
        vector.wait_ge(v_sem, 4)
        # 5: S = N * 2/s = Ns = R+I
        op("tensor_scalar", out=S, in0=S, scalar1=INV, scalar2=2.0,
           op0=Alu.mult, op1=Alu.mult)

        # ---- transform: out = X @ Ns + (t - X) via W + reduce ----
        W4 = W_t[:, :].rearrange("p (q n m) -> p q n m", q=Q, n=3)
        Xqn = PK[:, 0:21].rearrange("p (q n) -> p q n", n=3)
        Tb = T3.unsqueeze(1).broadcast_to([PARTS, Q, 3])
        # 6: D = t - X  -> W[..., 3]   (input-dep only)
        op("tensor_tensor", out=W4[:, :, :, 3], in0=Tb, in1=Xqn,
           op=Alu.subtract)
        # 7: W[q,n,m] = X[q,m] * Ns[m,n]
        Xb = Xqn.unsqueeze(2).broadcast_to([PARTS, Q, 3, 3])
        Sb = (S.rearrange("p (m n) -> p n m", m=3)
               .unsqueeze(1).broadcast_to([PARTS, Q, 3, 3]))
        vector.wait_ge(v_sem, 5)
        op("tensor_tensor", out=W4[:, :, :, 0:3], in0=Xb, in1=Sb,
           op=Alu.mult)
        # 8: O[q,n] = sum_m W[q,n,m]
        vector.wait_ge(v_sem, 7)
        vector.reduce_sum(
            out=O, in_=W_t[:, :].rearrange("p (qn m) -> p qn m", m=4),
            axis=mybir.AxisListType.X,
        ).then_inc(dve_sem, 1)

    return nc


def get_nc():
    if "nc" not in _cache:
        _cache["nc"] = _build_nc()
    return _cache["nc"]


def shard_inputs(pred_coor, r_vector, t_vector):
    n = pred_coor.shape[0]
    one = np.ones_like(r_vector[:, 0])
    b, c, d = r_vector[:, 0], r_vector[:, 1], r_vector[:, 2]
    pk = np.zeros((n, G, ROW), dtype=np.float32)
    pk[:, :, 0:21] = pred_coor.reshape(n, G, 21)
    pk[:, :, 21:24] = r_vector[:, None, :]
    a9 = (one, -d, c, d, one, -b, -c, b, one)
    for i, col in enumerate(a9):
        pk[:, :, 24 + i] = col[:, None]
    pk[:, :, 33:36] = t_vector[:, None, :]
    pk[:, :, 48] = 1.0
    pk = pk.reshape(n * G, ROW)
    return [
        {"packed": np.ascontiguousarray(pk[c * PARTS : (c + 1) * PARTS])}
        for c in range(NCORES)
    ]


def run(pred_coor, r_vector, t_vector, trace=False):
    from concourse.bass_utils import run_bass_kernel_spmd

    nc = get_nc()
    in_maps = shard_inputs(pred_coor, r_vector, t_vector)
    res = run_bass_kernel_spmd(nc, in_maps, list(range(NCORES)), trace=trace)
    full = np.concatenate([res.results[c]["out"] for c in range(NCORES)], axis=0)
    return full, res


def kernel(pred_coor, r_vector, t_vector):
    pred_coor = np.asarray(pred_coor, dtype=np.float32)
    r_vector = np.asarray(r_vector, dtype=np.float32)
    t_vector = np.asarray(t_vector, dtype=np.float32)
    full, _ = run(pred_coor, r_vector, t_vector, trace=False)
    return full


# revision 4
# speedup vs baseline: 1.1529x; 1.0155x over previous
"""Trainium2 Bass kernel for nn_Align — v13.

reference math (per structure j of 64):
    q = (1, b, c, d) / sqrt(s),  s = 1 + b^2 + c^2 + d^2
    R = rotmat(q);  out[j] = pred[j] @ R + t[j]

Sharding: data-parallel over the 8 NeuronCores, 8 structures per core.
Per-core layout: partitions = (structure j:8, group g:13) = 104, free dim
= (point q:7, coord n:3) = 21.

Formulation: with V = (b,c,d), numerators N (row-major S[3m+n]) obey
    N[m,n] = V[m]*V[n] + A[m,n],   A = [[1,-d,c],[d,1,-b],[-c,b,1]]
and with Ns = N*(2/s) = R+I:  out = X @ Ns + (t - X).
The product matrix is ONE broadcast outer-product op reading just 3 cols;
it lands in a PK scratch range whose stride-4 diagonal, together with a
host-packed 1.0 four columns past the last diag slot, forms the s-reduce
input (s = 1+bb+cc+dd) — no separate product packing needed.

8 DVE ops:
  1 P9 = V[m]*V[n] -> PK[36:45]   2 s = PK[32:48:4] = 1+bb+cc+dd
  3 INV = 1/s                     4 S = P9 + A9 = N
  5 S *= 2/s  (Ns = R+I)          6 D = t-X -> W[...,3]
  7 W[q,n,m] = X[q,m]*Ns[m,n]     8 O[q,n] = sum_m W[q,n,0:4]

Host packs per (j,g) row (36 floats — the DMA carries no scratch):
  [pred 21 | b c d | A9 = 1 -d c d 1 -b -c b 1 | t0 t1 t2]
The P9 outer-product scratch lives at SBUF cols 36:45, OUTSIDE the DMA'd
region (written by op1 before any read).  The s-reduce input (1,bb,cc,dd)
is the stride-4 run at cols 32,36,40,44 — A9's last 1.0 chains into the
P9 diagonal, so no separate 1.0 column is transferred.

Stale-semaphore safety (sems persist across NEFF runs): each sem is
cleared by its WAITER engine via an InstEventSemaphore write-0 before
that engine's first wait (DVE: dma_in+v, SP: dve+dma_out).  The framework
preamble ends in an all-engine barrier, so the clears are start-
synchronized, and every increment of a cleared sem is >=1us away (a DMA
completion or the whole DVE chain), so a clear cannot erase a live inc.
SP's clears sit after the input dma_start, hidden in its ~2.2us shadow.

No nc.Block(): no exit all-engine barrier.  SP's final wait on dma_out
keeps the NEFF alive until the output write lands; other engines idle.
Every DVE RAW dep is semaphore-synced (streaming same-engine RAW is not
safe on HW); deps transitively covered by an earlier-stalled wait carry
no encoded wait.
"""

import numpy as np

NCORES = 8
J = 8
G = 13
Q = 7
PARTS = J * G
ROW = 36

_cache = {}


def _build_nc():
    import dataclasses

    import concourse.bass as bass
    import concourse.mybir as mybir

    f32 = mybir.dt.float32
    Alu = mybir.AluOpType

    def restride(ap, dim_strides):
        pairs = [list(p) for p in ap.ap]
        for i, s in dim_strides.items():
            pairs[i][0] = s
        return dataclasses.replace(ap, ap=pairs)

    def sem_write0(engine, sem):
        # InstEventSemaphore: always-true wait carrying a write-to-0 update.
        # The natural sequencer encoding of "set semaphore := 0" on any engine.
        inst = engine.wait_ge(sem, 0)
        inst.ins.sync_info.on_update.append(
            mybir.SyncUpdate(
                sync_type="semaphore", id=sem.num, ant_name=sem.name,
                update_mode="sem-wr-imm", update_value=0,
            )
        )
        return inst

    nc = bass.Bass()
    packed = nc.dram_tensor("packed", [PARTS, ROW], f32, kind="ExternalInput")
    out = nc.dram_tensor("out", [J, 91, 3], f32, kind="ExternalOutput")

    with (
        nc.sbuf_tensor([PARTS, ROW + 9], f32) as PK_t,
        nc.sbuf_tensor([PARTS, 9], f32) as S_t,
        nc.sbuf_tensor([PARTS, 1], f32) as S2_t,
        nc.sbuf_tensor([PARTS, 1], f32) as INV_t,
        nc.sbuf_tensor([PARTS, Q * 3 * 4], f32) as W_t,
        nc.sbuf_tensor([PARTS, 21], f32) as O_t,
        nc.semaphore("dma_in") as dma_in_sem,
        nc.semaphore("v") as v_sem,
        nc.semaphore("dve_done") as dve_sem,
        nc.semaphore("dma_out") as dma_out_sem,
        nc.semaphore("pool_go") as pool_go_sem,
        nc.semaphore("pool_done") as pool_done_sem,
    ):
        PK = PK_t[:, :]
        S = S_t[:, :]
        S2 = S2_t[:, :]
        INV = INV_t[:, :]
        O = O_t[:, :]
        V3 = PK[:, 21:24]     # (b, c, d)
        A9 = PK[:, 24:33]     # 1 -d c d 1 -b -c b 1  (1.0s at 24, 28, 32)
        T3 = PK[:, 33:36]     # t
        P9 = PK[:, 36:45]     # outer-product scratch (NOT DMA'd; op1 writes)
        # s-reduce input: (1.0, bb, cc, dd) at cols 32, 36, 40, 44

        sync = nc.sync
        vector = nc.vector
        gpsimd = nc.gpsimd

        W4 = W_t[:, :].rearrange("p (q n m) -> p q n m", q=Q, n=3)
        Xqn = PK[:, 0:21].rearrange("p (q n) -> p q n", n=3)
        Tb = T3.unsqueeze(1).broadcast_to([PARTS, Q, 3])

        # ---- SP stream ----
        sync.dma_start(out=PK_t[:, 0:ROW], in_=packed[:, :]).then_inc(dma_in_sem, 16)
        sem_write0(sync, dve_sem)
        sem_write0(sync, dma_out_sem)
        sync.wait_ge(dve_sem, 1)
        sync.dma_start(
            out=out[:, :, :].rearrange("j (g q) m -> (j g) (q m)", g=G),
            in_=O,
        ).then_inc(dma_out_sem, 16)
        sync.wait_ge(dma_out_sem, 16)

        # ---- Pool stream: D = t - X -> W[..., 3], parallel to the DVE
        # quaternion chain.  Signals via its own pool_done sem: adding to the
        # shared v counter would let DVE's in-stream waits pass early and
        # re-expose the same-engine streaming-RAW hazard.
        sem_write0(gpsimd, pool_go_sem)
        gpsimd.wait_ge(pool_go_sem, 1)
        gpsimd.tensor_tensor(out=W4[:, :, :, 3], in0=Tb, in1=Xqn,
                             op=Alu.subtract).then_inc(pool_done_sem, 1)

        # ---- DVE stream ----
        sem_write0(vector, dma_in_sem)
        sem_write0(vector, v_sem)
        sem_write0(vector, pool_done_sem)
        vector.wait_ge(dma_in_sem, 16)
        # relay: input data is in SBUF -> release Pool's D op
        go = vector.wait_ge(v_sem, 0)
        go.ins.sync_info.on_update.append(
            mybir.SyncUpdate(
                sync_type="semaphore", id=pool_go_sem.num,
                ant_name=pool_go_sem.name,
                update_mode="sem-add-imm", update_value=1,
            )
        )

        def op(k, *args, **kw):
            return getattr(vector, k)(*args, **kw).then_inc(v_sem, 1)

        # 1: P9[3m+n] = V[m] * V[n]
        Vm = restride(V3.unsqueeze(2).broadcast_to([PARTS, 3, 3]), {2: 0})
        Vn = restride(V3.unsqueeze(1).broadcast_to([PARTS, 3, 3]), {1: 0})
        op("tensor_tensor", out=P9.rearrange("p (m n) -> p m n", m=3),
           in0=Vm, in1=Vn, op=Alu.mult)
        vector.wait_ge(v_sem, 1)
        # 2: s = 1 + bb + cc + dd  (stride-4 over PK cols 32,36,40,44)
        op("reduce_sum", out=S2, in_=restride(PK[:, 32:36], {1: 4}),
           axis=mybir.AxisListType.X)
        vector.wait_ge(v_sem, 2)
        # 3: INV = 1/s
        op("reciprocal", out=INV, in_=S2)
        # 4: S = P9 + A9 = N   (dep op1, covered by op2's wait)
        op("tensor_tensor", out=S, in0=P9, in1=A9, op=Alu.add)
        vector.wait_ge(v_sem, 4)
        # 5: S = N * 2/s = Ns = R+I
        op("tensor_scalar", out=S, in0=S, scalar1=INV, scalar2=2.0,
           op0=Alu.mult, op1=Alu.mult)

        # ---- transform: out = X @ Ns + (t - X) via W + reduce ----
        # (D = t-X runs on Pool, see above)
        # 6: W[q,n,m] = X[q,m] * Ns[m,n]
        Xb = Xqn.unsqueeze(2).broadcast_to([PARTS, Q, 3, 3])
        Sb = (S.rearrange("p (m n) -> p n m", m=3)
               .unsqueeze(1).broadcast_to([PARTS, Q, 3, 3]))
        vector.wait_ge(v_sem, 5)
        op("tensor_tensor", out=W4[:, :, :, 0:3], in0=Xb, in1=Sb,
           op=Alu.mult)
        # 7: O[q,n] = sum_m W[q,n,m]  (needs Pool's D AND the W product)
        vector.wait_ge(v_sem, 6)
        vector.wait_ge(pool_done_sem, 1)
        vector.reduce_sum(
            out=O, in_=W_t[:, :].rearrange("p (qn m) -> p qn m", m=4),
            axis=mybir.AxisListType.X,
        ).then_inc(dve_sem, 1)

    return nc


def get_nc():
    if "nc" not in _cache:
        _cache["nc"] = _build_nc()
    return _cache["nc"]


def shard_inputs(pred_coor, r_vector, t_vector):
    n = pred_coor.shape[0]
    one = np.ones_like(r_vector[:, 0])
    b, c, d = r_vector[:, 0], r_vector[:, 1], r_vector[:, 2]
    pk = np.empty((n, G, ROW), dtype=np.float32)
    pk[:, :, 0:21] = pred_coor.reshape(n, G, 21)
    pk[:, :, 21:24] = r_vector[:, None, :]
    a9 = (one, -d, c, d, one, -b, -c, b, one)
    for i, col in enumerate(a9):
        pk[:, :, 24 + i] = col[:, None]
    pk[:, :, 33:36] = t_vector[:, None, :]
    pk = pk.reshape(n * G, ROW)
    return [
        {"packed": np.ascontiguousarray(pk[c * PARTS : (c + 1) * PARTS])}
        for c in range(NCORES)
    ]


def run(pred_coor, r_vector, t_vector, trace=False):
    from concourse.bass_utils import run_bass_kernel_spmd

    nc = get_nc()
    in_maps = shard_inputs(pred_coor, r_vector, t_vector)
    res = run_bass_kernel_spmd(nc, in_maps, list(range(NCORES)), trace=trace)
    full = np.concatenate([res.results[c]["out"] for c in range(NCORES)], axis=0)
    return full, res


def kernel(pred_coor, r_vector, t_vector):
    pred_coor = np.asarray(pred_coor, dtype=np.float32)
    r_vector = np.asarray(r_vector, dtype=np.float32)
    t_vector = np.asarray(t_vector, dtype=np.float32)
    full, _ = run(pred_coor, r_vector, t_vector, trace=False)
    return full
